# revision 2
# baseline (speedup 1.0000x reference)
"""Trainium2 Bass kernel: single-head causal self-attention.

Problem: B=4, S=2048, D=1024, f32 IO.
  Q = x@Wq + bq; K = x@Wk + bk; V = x@Wv + bv
  out = softmax(causal(Q K^T / sqrt(D))) @ V

Sharding over 8 NeuronCores: core c -> (batch b = c//2, part p = c%2).
Each batch's 2048 query rows are split into eight 256-row windows
W_0..W_7 (causal k-tile depth of W_m is 2m+2, in 128-key tiles).
p=0 takes windows [0,3,4,7] (depths 2,8,10,16), p=1 takes [1,2,5,6]
(depths 4,6,12,14) -- both sum to 36 so attention work is balanced.
The SPMD graph is identical on every core: it processes 4 "slots" with
fixed k-depths L=[4,8,12,16]; which global window sits in which slot is
pure data placement (host permutes Q rows / output rows, and boundary
causal masks for the last 4 k-tiles of each slot are passed as inputs).

On-device layout: everything is consumed via x^T [d, s], so the host
passes x^T/xq^T (bf16) directly.  Projections produce K^T,Q^T [d,s] and
V [s,d]; attention computes scores^T [k,q] so that softmax sums reduce
via a ones-matmul and attn@V needs no transposes.  Matmuls in bf16,
softmax/normalization in f32.
"""

import sys

import numpy as np

if "/opt/trn_rl_repo" not in sys.path:
    sys.path.insert(0, "/opt/trn_rl_repo")

import ml_dtypes

D = 1024
S = 2048
B = 4
P = 128
NCORES = 8
WINDOWS = {0: [0, 3, 4, 7], 1: [1, 2, 5, 6]}
LDEPTH = [4, 8, 12, 16]
BF16 = ml_dtypes.bfloat16

_GRAPH = None


def _build_graph():
    import concourse.bass as bass  # noqa: F401
    from concourse import bacc, mybir, tile

    f32 = mybir.dt.float32
    b16 = mybir.dt.bfloat16

    nc = bacc.Bacc(None, target_bir_lowering=False, debug=False)

    xt_d = nc.declare_dram_parameter("xt", [D, S], b16, False)
    xqt_d = nc.declare_dram_parameter("xqt", [D, 1024], b16, False)
    wq_d = nc.declare_dram_parameter("wq", [D, D], b16, False)
    wk_d = nc.declare_dram_parameter("wk", [D, D], b16, False)
    wv_d = nc.declare_dram_parameter("wv", [D, D], b16, False)
    bq_d = nc.declare_dram_parameter("bq", [P, 8], f32, False)
    bk_d = nc.declare_dram_parameter("bk", [P, 8], f32, False)
    bvb_d = nc.declare_dram_parameter("bvb", [P, D], f32, False)
    mk_d = nc.declare_dram_parameter("masks", [16, P, 256], b16, False)
    out_d = nc.declare_dram_parameter("out", [1024, D], f32, True)

    with tile.TileContext(nc) as tc:
        with (
            tc.tile_pool(name="const", bufs=1) as const,
            tc.tile_pool(name="psA", bufs=2, space="PSUM") as psA,
            tc.tile_pool(name="psB", bufs=4, space="PSUM") as psB,
            tc.tile_pool(name="psS", bufs=2, space="PSUM") as psS,
            tc.tile_pool(name="evict", bufs=3) as evict,
        ):
            xt = const.tile([P, 8, S], b16, name="xt_s")
            xqt = const.tile([P, 8, 1024], b16, name="xqt_s")
            w_sb = {
                n: const.tile([P, 8, D], b16, name=f"w_{n}_s") for n in ("q", "k", "v")
            }
            kT = const.tile([P, 8, S], b16, name="kT_s")
            qT = const.tile([P, 8, 1024], b16, name="qT_s")
            vv = const.tile([P, 16, D], b16, name="v_s")
            bq_s = const.tile([P, 8], f32, name="bq_s")
            bk_s = const.tile([P, 8], f32, name="bk_s")
            bvb_s = const.tile([P, D], f32, name="bvb_s")
            mk_s = const.tile([P, 16, 256], b16, name="mk_s")
            ones_s = const.tile([P, 1], b16, name="ones_s")

            nc.sync.dma_start(xt[:], xt_d.ap().rearrange("(o p) s -> p o s", p=P))
            nc.sync.dma_start(xqt[:], xqt_d.ap().rearrange("(o p) s -> p o s", p=P))
            for n, dram in (("q", wq_d), ("k", wk_d), ("v", wv_d)):
                nc.sync.dma_start(
                    w_sb[n][:], dram.ap().rearrange("(o p) e -> p o e", p=P)
                )
            nc.sync.dma_start(bq_s[:], bq_d.ap())
            nc.sync.dma_start(bk_s[:], bk_d.ap())
            nc.sync.dma_start(bvb_s[:], bvb_d.ap())
            nc.sync.dma_start(mk_s[:], mk_d.ap().rearrange("n p f -> p n f"))
            nc.any.memset(ones_s[:], 1.0)

            ident = mybir.ActivationFunctionType.Identity

            # K^T and Q^T projections: psum[d_out 128, s 512] = sum_di W[di,e].T @ xT[di,s]
            for wname, src, dst, ncols, bias in (
                ("k", xt, kT, S, bk_s),
                ("q", xqt, qT, 1024, bq_s),
            ):
                for et in range(8):
                    for sw in range(ncols // 512):
                        ps = psA.tile([P, 512], f32, name="ps_mm")
                        for di in range(8):
                            nc.tensor.matmul(
                                ps[:],
                                w_sb[wname][:, di, et * P : (et + 1) * P],
                                src[:, di, sw * 512 : (sw + 1) * 512],
                                start=(di == 0),
                                stop=(di == 7),
                            )
                        nc.scalar.activation(
                            dst[:, et, sw * 512 : (sw + 1) * 512],
                            ps[:],
                            ident,
                            bias=bias[:, et : et + 1],
                            scale=1.0,
                        )

            # V projection: psum[s 128, d_out 512] = sum_di xT[di,s].T @ Wv[di,d]
            for st in range(16):
                for dw in range(2):
                    ps = psA.tile([P, 512], f32, name="ps_mm")
                    for di in range(8):
                        nc.tensor.matmul(
                            ps[:],
                            xt[:, di, st * P : (st + 1) * P],
                            w_sb["v"][:, di, dw * 512 : (dw + 1) * 512],
                            start=(di == 0),
                            stop=(di == 7),
                        )
                    nc.vector.tensor_tensor(
                        vv[:, st, dw * 512 : (dw + 1) * 512],
                        ps[:],
                        bvb_s[:, dw * 512 : (dw + 1) * 512],
                        mybir.AluOpType.add,
                    )

            # Attention: per slot, scores^T [k 128, q 256] per k-tile,
            # exp (f32 psum -> bf16), boundary mask, then
            # O_unnorm[q,d] += expS^T.T @ V and sums[q] += expS^T.T @ ones.
            inv_sqrt_d = float(1.0 / np.sqrt(D))
            exp_f = mybir.ActivationFunctionType.Exp
            for slot in range(4):
                L = LDEPTH[slot]
                q0 = 256 * slot
                pO = [psB.tile([P, 512], mybir.dt.float32, name="psO") for _ in range(4)]
                pSm = [psS.tile([P, 1], mybir.dt.float32, name="psSum") for _ in range(2)]
                for kt in range(L):
                    ps = psA.tile([P, 256], mybir.dt.float32, name="ps_mm")
                    for di in range(8):
                        nc.tensor.matmul(
                            ps[:, :256],
                            kT[:, di, kt * P : (kt + 1) * P],
                            qT[:, di, q0 : q0 + 256],
                            start=(di == 0),
                            stop=(di == 7),
                        )
                    eS = evict.tile([P, 256], b16, name="eS")
                    nc.scalar.activation(eS[:], ps[:, :256], exp_f, scale=inv_sqrt_d)
                    if kt >= L - 4:
                        nc.vector.tensor_tensor(
                            eS[:],
                            eS[:],
                            mk_s[:, slot * 4 + (kt - (L - 4)), :],
                            mybir.AluOpType.mult,
                        )
                    for jj in range(2):
                        st_op = eS[:, jj * P : (jj + 1) * P]
                        nc.tensor.matmul(
                            pSm[jj][:],
                            st_op,
                            ones_s[:],
                            start=(kt == 0),
                            stop=(kt == L - 1),
                        )
                        for half in range(2):
                            nc.tensor.matmul(
                                pO[jj * 2 + half][:],
                                st_op,
                                vv[:, kt, half * 512 : (half + 1) * 512],
                                start=(kt == 0),
                                stop=(kt == L - 1),
                            )
                for jj in range(2):
                    rec = evict.tile([P, 1], mybir.dt.float32, name="recip")
                    nc.vector.reciprocal(rec[:], pSm[jj][:])
                    for half in range(2):
                        o_sb = evict.tile([P, 512], mybir.dt.float32, name="o_sb")
                        nc.vector.tensor_scalar_mul(o_sb[:], pO[jj * 2 + half][:], rec[:])
                        nc.sync.dma_start(
                            out_d.ap()[
                                q0 + jj * P : q0 + (jj + 1) * P,
                                half * 512 : (half + 1) * 512,
                            ],
                            o_sb[:],
                        )

    nc.compile()
    return nc


def _get_graph():
    global _GRAPH
    if _GRAPH is None:
        _GRAPH = _build_graph()
    return _GRAPH


def _masks_for(p):
    masks = np.zeros((16, P, 256), dtype=BF16)
    k_idx = np.arange(P)[:, None]
    q_idx = np.arange(256)[None, :]
    for slot in range(4):
        L = LDEPTH[slot]
        m = WINDOWS[p][slot]
        for r in range(4):
            kt = L - 4 + r
            valid = (kt * P + k_idx) <= (256 * m + q_idx)
            masks[slot * 4 + r] = valid.astype(BF16)
    return masks


def _make_in_maps(x, Wq, bq, Wk, bk, Wv, bv):
    x = np.asarray(x, dtype=np.float32)
    wq_b = np.asarray(Wq, dtype=np.float32).astype(BF16)
    wk_b = np.asarray(Wk, dtype=np.float32).astype(BF16)
    wv_b = np.asarray(Wv, dtype=np.float32).astype(BF16)
    bq2 = np.ascontiguousarray(
        np.asarray(bq, np.float32).reshape(8, P).T
    )
    bk2 = np.ascontiguousarray(
        np.asarray(bk, np.float32).reshape(8, P).T
    )
    bvb = np.ascontiguousarray(
        np.broadcast_to(np.asarray(bv, np.float32), (P, D))
    )
    masks_by_p = {p: _masks_for(p) for p in (0, 1)}
    in_maps = []
    for c in range(NCORES):
        b, p = divmod(c, 2)
        xt = np.ascontiguousarray(x[b].T).astype(BF16)
        qrows = np.concatenate(
            [x[b][256 * m : 256 * (m + 1)] for m in WINDOWS[p]], axis=0
        )
        xqt = np.ascontiguousarray(qrows.T).astype(BF16)
        in_maps.append(
            dict(
                xt=xt,
                xqt=xqt,
                wq=wq_b,
                wk=wk_b,
                wv=wv_b,
                bq=bq2,
                bk=bk2,
                bvb=bvb,
                masks=masks_by_p[p],
            )
        )
    return in_maps


def _assemble(results):
    out = np.empty((B, S, D), dtype=np.float32)
    for c in range(NCORES):
        b, p = divmod(c, 2)
        o = results[c]["out"]
        for slot, m in enumerate(WINDOWS[p]):
            out[b, 256 * m : 256 * (m + 1)] = o[256 * slot : 256 * (slot + 1)]
    return out


def _run(in_maps, trace=False, **kwargs):
    from concourse.bass_utils import run_bass_kernel_spmd

    nc = _get_graph()
    return run_bass_kernel_spmd(
        nc, in_maps, core_ids=list(range(NCORES)), trace=trace, **kwargs
    )


def kernel(x, Wq, bq, Wk, bk, Wv, bv):
    in_maps = _make_in_maps(x, Wq, bq, Wk, bk, Wv, bv)
    res = _run(in_maps)
    return _assemble(res.results)


def _install_profile_shim():
    """The agent image's ``antenv`` lacks ``axon_hooks``; recreate it so
    run_bass_kernel_spmd(trace=True) can find the NTFF profile hook, and
    stub out the artifact upload (no bucket access here)."""
    import types

    if "antenv.axon_hooks" not in sys.modules:
        mod = types.ModuleType("antenv.axon_hooks")
        mod._hook = None

        def set_axon_ntff_profile_hook(h):
            mod._hook = h

        def get_axon_ntff_profile_hook():
            return mod._hook

        mod.set_axon_ntff_profile_hook = set_axon_ntff_profile_hook
        mod.get_axon_ntff_profile_hook = get_axon_ntff_profile_hook
        sys.modules["antenv.axon_hooks"] = mod

    if sys.modules["antenv.axon_hooks"]._hook is None:
        from trn_agent_boot.trn_boot import _ntff_profile_via_ctypes

        sys.modules["antenv.axon_hooks"].set_axon_ntff_profile_hook(
            _ntff_profile_via_ctypes("/opt/axon/libaxon_pjrt.so")
        )

    from concourse import bass_utils

    bass_utils.upload_artifacts = lambda tmpdir: f"local:{tmpdir}"


def profile(inputs, **kwargs):
    """Run with tracing; returns (exec_time_ns, BassKernelResults)."""
    _install_profile_shim()
    in_maps = _make_in_maps(**inputs)
    res = _run(in_maps, trace=True, **kwargs)
    return res.exec_time_ns, res


# revision 7
# speedup vs baseline: 1.0034x; 1.0034x over previous
"""Trainium2 Bass kernel: single-head causal self-attention.

Problem: B=4, S=2048, D=1024, f32 IO.
  Q = x@Wq + bq; K = x@Wk + bk; V = x@Wv + bv
  out = softmax(causal(Q K^T / sqrt(D))) @ V

Sharding over 8 NeuronCores: core c -> (batch b = c//2, part p = c%2).
Each batch's 2048 query rows are split into eight 256-row windows
W_0..W_7 (causal k-tile depth of W_m is 2m+2, in 128-key tiles).
p=0 takes windows [0,3,4,7] (depths 2,8,10,16), p=1 takes [1,2,5,6]
(depths 4,6,12,14) -- both sum to 36 so attention work is balanced.
The SPMD graph is identical on every core: it processes 4 "slots" with
fixed k-depths L=[4,8,12,16]; which global window sits in which slot is
pure data placement (host permutes Q rows / output rows, and boundary
causal masks for the last 4 k-tiles of each slot are passed as inputs).

K and V projections are split across the core pair: each core projects
its 1024-key half (from its half of x^T, pre-permuted so own-half is
canonical), then the pair exchanges K^T/V halves with an AllGather over
replica groups [[0,1],[2,3],[4,5],[6,7]] through DRAM bounce buffers.

On-device layout: everything is consumed via x^T [d, s], so the host
passes x^T/xq^T (bf16) directly.  Projections produce K^T,Q^T [d,s] and
V [s,d]; attention computes scores^T [k,q] so that softmax sums reduce
via a ones-matmul and attn@V needs no transposes.  Matmuls in bf16,
softmax/normalization in f32.
"""

import sys

import numpy as np

if "/opt/trn_rl_repo" not in sys.path:
    sys.path.insert(0, "/opt/trn_rl_repo")

import ml_dtypes

D = 1024
S = 2048
B = 4
P = 128
NCORES = 8
WINDOWS = {0: [0, 3, 4, 7], 1: [1, 2, 5, 6]}
LDEPTH = [4, 8, 12, 16]
BF16 = ml_dtypes.bfloat16

_GRAPH = None


def _build_graph():
    import concourse.bass as bass  # noqa: F401
    from concourse import bacc, mybir, tile

    f32 = mybir.dt.float32
    b16 = mybir.dt.bfloat16

    nc = bacc.Bacc(None, target_bir_lowering=False, debug=False, num_devices=NCORES)

    # own-half x^T for K/V projections (columns = own 1024 keys, global order
    # position is p*1024 but canonical 0:1024 here)
    xt_d = nc.declare_dram_parameter("xt", [D, 1024], b16, False)
    xqt_d = nc.declare_dram_parameter("xqt", [D, 1024], b16, False)
    wq_d = nc.declare_dram_parameter("wq", [D, D], b16, False)
    wk_d = nc.declare_dram_parameter("wk", [D, D], b16, False)
    wv_d = nc.declare_dram_parameter("wv", [D, D], b16, False)
    bq_d = nc.declare_dram_parameter("bq", [P, 8], f32, False)
    bk_d = nc.declare_dram_parameter("bk", [P, 8], f32, False)
    bvb_d = nc.declare_dram_parameter("bvb", [P, D], f32, False)
    mk_d = nc.declare_dram_parameter("masks", [16, P, 256], b16, False)
    out_d = nc.declare_dram_parameter("out", [1024, D], f32, True)

    # collective bounce buffers ([gathered] outputs Shared for perf)
    ksend = nc.dram_tensor("ksend", [1024, 1024], b16)
    krecv = nc.dram_tensor("krecv", [2048, 1024], b16)
    vsend = nc.dram_tensor("vsend", [1024, 1024], b16)
    vrecv = nc.dram_tensor("vrecv", [2048, 1024], b16)
    groups = [[0, 1], [2, 3], [4, 5], [6, 7]]

    with tile.TileContext(nc) as tc:
        with (
            tc.tile_pool(name="const", bufs=1) as const,
            tc.tile_pool(name="psA", bufs=2, space="PSUM") as psA,
            tc.tile_pool(name="psB", bufs=4, space="PSUM") as psB,
            tc.tile_pool(name="psS", bufs=2, space="PSUM") as psS,
            tc.tile_pool(name="evict", bufs=3) as evict,
        ):
            xt = const.tile([P, 8, 1024], b16, name="xt_s")
            xqt = const.tile([P, 8, 1024], b16, name="xqt_s")
            w_sb = {
                n: const.tile([P, 8, D], b16, name=f"w_{n}_s") for n in ("q", "k", "v")
            }
            kT = const.tile([P, 8, S], b16, name="kT_s")
            qT = const.tile([P, 8, 1024], b16, name="qT_s")
            vv = const.tile([P, 16, D], b16, name="v_s")
            bq_s = const.tile([P, 8], f32, name="bq_s")
            bk_s = const.tile([P, 8], f32, name="bk_s")
            bvb_s = const.tile([P, D], f32, name="bvb_s")
            mk_s = const.tile([P, 16, 256], b16, name="mk_s")
            ones_s = const.tile([P, 1], b16, name="ones_s")

            # Input DMAs, chunked and ordered so the K-projection's first
            # matmuls unblock after ~1.5MB instead of after all inputs.
            def load_striped(dst, dram, what="(o p) s -> p o s"):
                v = dram.ap().rearrange(what, p=P)
                for c in range(2):
                    sl = slice(c * 512, (c + 1) * 512)
                    nc.sync.dma_start(dst[:, :, sl], v[:, :, sl])

            nc.sync.dma_start(bk_s[:], bk_d.ap())
            load_striped(w_sb["k"], wk_d)
            load_striped(xt, xt_d)
            nc.sync.dma_start(bvb_s[:], bvb_d.ap())
            load_striped(w_sb["v"], wv_d)
            nc.sync.dma_start(bq_s[:], bq_d.ap())
            load_striped(w_sb["q"], wq_d)
            load_striped(xqt, xqt_d)
            nc.sync.dma_start(mk_s[:], mk_d.ap().rearrange("n p f -> p n f"))
            nc.any.memset(ones_s[:], 1.0)

            ident = mybir.ActivationFunctionType.Identity

            # K^T projection (own 1024 keys):
            # psum[d_out 128, s 512] = sum_di Wk[di,e].T @ xT[di,s]
            for sw in range(2):
                for et in range(8):
                    ps = psA.tile([P, 512], f32, name="ps_mm")
                    for di in range(8):
                        nc.tensor.matmul(
                            ps[:],
                            w_sb["k"][:, di, et * P : (et + 1) * P],
                            xt[:, di, sw * 512 : (sw + 1) * 512],
                            start=(di == 0),
                            stop=(di == 7),
                        )
                    ko = evict.tile([P, 512], b16, name="ko")
                    nc.scalar.activation(
                        ko[:],
                        ps[:],
                        ident,
                        bias=bk_s[:, et : et + 1],
                        scale=1.0,
                    )
                    nc.sync.dma_start(
                        ksend.ap()[
                            et * P : (et + 1) * P, sw * 512 : (sw + 1) * 512
                        ],
                        ko[:],
                    )
            nc.gpsimd.collective_compute(
                "AllGather",
                mybir.AluOpType.bypass,
                replica_groups=groups,
                ins=[ksend.ap().opt()],
                outs=[krecv.ap().opt()],
            )
            for g in range(2):
                nc.sync.dma_start(
                    kT[:, :, g * 1024 : (g + 1) * 1024],
                    krecv.ap()[g * 1024 : (g + 1) * 1024].rearrange(
                        "(o p) s -> p o s", p=P
                    ),
                )

            # V projection (own 1024 keys):
            # psum[s 128, d_out 512] = sum_di xT[di,s].T @ Wv[di,d]
            for st in range(8):
                for dw in range(2):
                    ps = psA.tile([P, 512], f32, name="ps_mm")
                    for di in range(8):
                        nc.tensor.matmul(
                            ps[:],
                            xt[:, di, st * P : (st + 1) * P],
                            w_sb["v"][:, di, dw * 512 : (dw + 1) * 512],
                            start=(di == 0),
                            stop=(di == 7),
                        )
                    vo = evict.tile([P, 512], b16, name="vo")
                    nc.vector.tensor_tensor(
                        vo[:],
                        ps[:],
                        bvb_s[:, dw * 512 : (dw + 1) * 512],
                        mybir.AluOpType.add,
                    )
                    nc.sync.dma_start(
                        vsend.ap()[
                            st * P : (st + 1) * P, dw * 512 : (dw + 1) * 512
                        ],
                        vo[:],
                    )
            nc.gpsimd.collective_compute(
                "AllGather",
                mybir.AluOpType.bypass,
                replica_groups=groups,
                ins=[vsend.ap().opt()],
                outs=[vrecv.ap().opt()],
            )
            for g in range(2):
                nc.sync.dma_start(
                    vv[:, g * 8 : (g + 1) * 8, :],
                    vrecv.ap()[g * 1024 : (g + 1) * 1024].rearrange(
                        "(o p) d -> p o d", p=P
                    ),
                )

            # Q^T projection (this core's 1024 query rows)
            for et in range(8):
                for sw in range(2):
                    ps = psA.tile([P, 512], f32, name="ps_mm")
                    for di in range(8):
                        nc.tensor.matmul(
                            ps[:],
                            w_sb["q"][:, di, et * P : (et + 1) * P],
                            xqt[:, di, sw * 512 : (sw + 1) * 512],
                            start=(di == 0),
                            stop=(di == 7),
                        )
                    nc.scalar.activation(
                        qT[:, et, sw * 512 : (sw + 1) * 512],
                        ps[:],
                        ident,
                        bias=bq_s[:, et : et + 1],
                        scale=1.0,
                    )

            # Attention: per slot, scores^T [k 128, q 256] per k-tile,
            # exp (f32 psum -> bf16), boundary mask, then
            # O_unnorm[q,d] += expS^T.T @ V and sums[q] += expS^T.T @ ones.
            inv_sqrt_d = float(1.0 / np.sqrt(D))
            exp_f = mybir.ActivationFunctionType.Exp
            for slot in range(4):
                L = LDEPTH[slot]
                q0 = 256 * slot
                pO = [psB.tile([P, 512], f32, name="psO") for _ in range(4)]
                pSm = [psS.tile([P, 1], f32, name="psSum") for _ in range(2)]
                for kt in range(L):
                    ps = psA.tile([P, 256], f32, name="ps_mm")
                    for di in range(8):
                        nc.tensor.matmul(
                            ps[:, :256],
                            kT[:, di, kt * P : (kt + 1) * P],
                            qT[:, di, q0 : q0 + 256],
                            start=(di == 0),
                            stop=(di == 7),
                        )
                    eS = evict.tile([P, 256], b16, name="eS")
                    nc.scalar.activation(eS[:], ps[:, :256], exp_f, scale=inv_sqrt_d)
                    if kt >= L - 4:
                        nc.vector.tensor_tensor(
                            eS[:],
                            eS[:],
                            mk_s[:, slot * 4 + (kt - (L - 4)), :],
                            mybir.AluOpType.mult,
                        )
                    for jj in range(2):
                        st_op = eS[:, jj * P : (jj + 1) * P]
                        nc.tensor.matmul(
                            pSm[jj][:],
                            st_op,
                            ones_s[:],
                            start=(kt == 0),
                            stop=(kt == L - 1),
                        )
                        for half in range(2):
                            nc.tensor.matmul(
                                pO[jj * 2 + half][:],
                                st_op,
                                vv[:, kt, half * 512 : (half + 1) * 512],
                                start=(kt == 0),
                                stop=(kt == L - 1),
                            )
                for jj in range(2):
                    rec = evict.tile([P, 1], f32, name="recip")
                    nc.vector.reciprocal(rec[:], pSm[jj][:])
                    for half in range(2):
                        o_sb = evict.tile([P, 512], f32, name="o_sb")
                        nc.vector.tensor_scalar_mul(o_sb[:], pO[jj * 2 + half][:], rec[:])
                        nc.sync.dma_start(
                            out_d.ap()[
                                q0 + jj * P : q0 + (jj + 1) * P,
                                half * 512 : (half + 1) * 512,
                            ],
                            o_sb[:],
                        )

    nc.compile()
    return nc


def _get_graph():
    global _GRAPH
    if _GRAPH is None:
        _GRAPH = _build_graph()
    return _GRAPH


def _masks_for(p):
    masks = np.zeros((16, P, 256), dtype=BF16)
    k_idx = np.arange(P)[:, None]
    q_idx = np.arange(256)[None, :]
    for slot in range(4):
        L = LDEPTH[slot]
        m = WINDOWS[p][slot]
        for r in range(4):
            kt = L - 4 + r
            valid = (kt * P + k_idx) <= (256 * m + q_idx)
            masks[slot * 4 + r] = valid.astype(BF16)
    return masks


def _make_in_maps(x, Wq, bq, Wk, bk, Wv, bv):
    x = np.asarray(x, dtype=np.float32)
    wq_b = np.asarray(Wq, dtype=np.float32).astype(BF16)
    wk_b = np.asarray(Wk, dtype=np.float32).astype(BF16)
    wv_b = np.asarray(Wv, dtype=np.float32).astype(BF16)
    bq2 = np.ascontiguousarray(np.asarray(bq, np.float32).reshape(8, P).T)
    bk2 = np.ascontiguousarray(np.asarray(bk, np.float32).reshape(8, P).T)
    bvb = np.ascontiguousarray(np.broadcast_to(np.asarray(bv, np.float32), (P, D)))
    masks_by_p = {p: _masks_for(p) for p in (0, 1)}
    in_maps = []
    for c in range(NCORES):
        b, p = divmod(c, 2)
        xT = x[b].T.astype(BF16)
        xt = np.ascontiguousarray(xT[:, p * 1024 : (p + 1) * 1024])
        qcols = np.concatenate(
            [xT[:, 256 * m : 256 * (m + 1)] for m in WINDOWS[p]], axis=1
        )
        xqt = np.ascontiguousarray(qcols)
        in_maps.append(
            dict(
                xt=xt,
                xqt=xqt,
                wq=wq_b,
                wk=wk_b,
                wv=wv_b,
                bq=bq2,
                bk=bk2,
                bvb=bvb,
                masks=masks_by_p[p],
            )
        )
    return in_maps


def _assemble(results):
    out = np.empty((B, S, D), dtype=np.float32)
    for c in range(NCORES):
        b, p = divmod(c, 2)
        o = results[c]["out"]
        for slot, m in enumerate(WINDOWS[p]):
            out[b, 256 * m : 256 * (m + 1)] = o[256 * slot : 256 * (slot + 1)]
    return out


def _run(in_maps, trace=False, **kwargs):
    from concourse.bass_utils import run_bass_kernel_spmd

    nc = _get_graph()
    return run_bass_kernel_spmd(
        nc, in_maps, core_ids=list(range(NCORES)), trace=trace, **kwargs
    )


def kernel(x, Wq, bq, Wk, bk, Wv, bv):
    in_maps = _make_in_maps(x, Wq, bq, Wk, bk, Wv, bv)
    res = _run(in_maps)
    return _assemble(res.results)


def _install_profile_shim():
    """The agent image's ``antenv`` lacks ``axon_hooks``; recreate it so
    run_bass_kernel_spmd(trace=True) can find the NTFF profile hook, and
    stub out the artifact upload (no bucket access here)."""
    import types

    if "antenv.axon_hooks" not in sys.modules:
        mod = types.ModuleType("antenv.axon_hooks")
        mod._hook = None

        def set_axon_ntff_profile_hook(h):
            mod._hook = h

        def get_axon_ntff_profile_hook():
            return mod._hook

        mod.set_axon_ntff_profile_hook = set_axon_ntff_profile_hook
        mod.get_axon_ntff_profile_hook = get_axon_ntff_profile_hook
        sys.modules["antenv.axon_hooks"] = mod

    if sys.modules["antenv.axon_hooks"]._hook is None:
        from trn_agent_boot.trn_boot import _ntff_profile_via_ctypes

        sys.modules["antenv.axon_hooks"].set_axon_ntff_profile_hook(
            _ntff_profile_via_ctypes("/opt/axon/libaxon_pjrt.so")
        )

    from concourse import bass_utils

    bass_utils.upload_artifacts = lambda tmpdir: f"local:{tmpdir}"


def profile(inputs, **kwargs):
    """Run with tracing; returns (exec_time_ns, BassKernelResults)."""
    _install_profile_shim()
    in_maps = _make_in_maps(**inputs)
    res = _run(in_maps, trace=True, **kwargs)
    return res.exec_time_ns, res
